# revision 1
# baseline (speedup 1.0000x reference)
"""CRF negative-log-likelihood loss (BERT_BiLSTM_CRF) on 8 TRN2 NeuronCores.

Strategy (data-parallel over batch, 64 sequences/core):
 - Linear-space forward algorithm with the 32x32 exp(transitions) matrix as
   PE matmul weights (block-diag [fwd | bwd]); per step one matmul + one DVE
   elementwise multiply by exp(emissions - MU).
 - Forward and backward (meet-in-the-middle) chains run in the same per-tick
   matmul, halving the serial step count to 1024.
 - Emissions are streamed in big DMA chunks, transposed to [tag, seq] layout
   on the PE, and exponentiated PSUM->SBUF on the scalar engine.
 - Periodic renorm (every 128 ticks) by a proxy-row reciprocal keeps fp32 in
   range; log(scale) accumulates into the per-sequence offset.
 - Gold-path score via two indirect (gathering) DMAs + free-dim reduces.
"""
import numpy as np

TAGSET = 32
START = 30
STOP = 31
B = 512
S = 2048
NCORES = 8
BC = B // NCORES          # 64 sequences per core
HALF = S // 2             # 1024 ticks per direction
CH = 64                   # emission steps per streamed chunk
NCH = HALF // CH          # 16 chunks per direction
MU = np.float32(4.3226)   # mean log-growth per step (measured offline)
REN = 128                 # renorm period in ticks

_CACHE = {}


def _build_nc(debug=False, gold=2):
    import concourse.bacc as bacc
    import concourse.bass as bass
    import concourse.tile as tile
    from concourse import mybir

    f32 = mybir.dt.float32
    i32 = mybir.dt.int32
    AF = mybir.ActivationFunctionType
    OP = mybir.AluOpType
    AX = mybir.AxisListType

    nc = bacc.Bacc("TRN2", target_bir_lowering=False, debug=False,
                   num_devices=NCORES)

    em_d = nc.dram_tensor("emissions", [BC, S, TAGSET], f32,
                          kind="ExternalInput").ap()
    tg_d = nc.dram_tensor("tags", [BC, S], i32, kind="ExternalInput").ap()
    tr_d = nc.dram_tensor("transitions", [TAGSET, TAGSET], f32,
                          kind="ExternalInput").ap()
    nll_d = nc.dram_tensor("nll", [1, BC], f32, kind="ExternalOutput").ap()
    dbg = {}
    if debug:
        for name in ["d_gold", "d_st0", "d_st128", "d_xt0", "d_w"]:
            dbg[name] = nc.dram_tensor(name, [64, 64], f32,
                                       kind="ExternalOutput").ap()

    with tile.TileContext(nc) as tc:
        with (
            tc.tile_pool(name="const", bufs=1) as cp,
            tc.tile_pool(name="chunk", bufs=3) as ccp,
            tc.tile_pool(name="oh", bufs=2) as ohp,
            tc.tile_pool(name="xt", bufs=6) as xtp,
            tc.tile_pool(name="state", bufs=3) as stp,
            tc.tile_pool(name="small", bufs=2) as smp,
            tc.tile_pool(name="trp", bufs=3, space="PSUM") as trp,
            tc.tile_pool(name="mmp", bufs=2, space="PSUM") as mmp,
            tc.tile_pool(name="finp", bufs=1, space="PSUM") as fip,
        ):
            # ---------------- setup: weights, identity, ones ----------------
            w = cp.tile([64, 64], f32)
            nc.vector.memset(w[:], 0.0)
            # fwd block: w[p, t] = trans[t, p]  (strided transpose DMA, tiny)
            nc.sync.dma_start(w[0:32, 0:32], tr_d.rearrange("a b -> b a"))
            # bwd block: w[32+p, 32+t] = trans[p, t]
            nc.sync.dma_start(w[32:64, 32:64], tr_d)
            # clamp -1e4 entries so the exp LUT stays in-range, then exp
            nc.vector.tensor_scalar_max(w[:], w[:], -80.0)
            nc.scalar.activation(w[:], w[:], AF.Exp)
            # zero the off-diagonal blocks again (exp(0)=1 crept in)
            nc.vector.memset(w[0:32, 32:64], 0.0)
            nc.vector.memset(w[32:64, 0:32], 0.0)

            ones_t = cp.tile([64, 64], f32)
            nc.vector.memset(ones_t[:], 1.0)
            negmu = cp.tile([64, 1], f32)
            nc.vector.memset(negmu[:], -float(MU))
            ident = cp.tile([64, 64], f32)
            nc.gpsimd.affine_select(
                out=ident[:], in_=ones_t[:], pattern=[[-1, 64]],
                compare_op=OP.is_equal, fill=0.0, base=0, channel_multiplier=1)

            # ---------------- gold emission score (one-hot) ----------------
            # e_score[b] = sum_s em[b, s, tags[b, s]]  computed per chunk as
            # sum((t-iota == tag) * em) with DVE is_equal + mult-reduce.
            # The transition part of the gold score is tiny (tags x 32x32
            # table) and is folded in on the host during unshard.
            tags_sb = cp.tile([BC, S], i32)
            nc.sync.dma_start(tags_sb[:], tg_d)
            iota_t = cp.tile([BC, CH * TAGSET], i32)
            nc.gpsimd.iota(iota_t[:], pattern=[[0, CH], [1, TAGSET]], base=0,
                           channel_multiplier=0)
            acc_e = cp.tile([BC, NCH], f32)
            nc.vector.memset(acc_e[:], 0.0)
            if debug:
                nc.sync.dma_start(dbg["d_gold"][:, 0:1], gold_c[:])

            # ---------------- scan state init ----------------
            offacc = cp.tile([64, 64], f32)
            nc.vector.memset(offacc[:], 0.0)

            state = stp.tile([64, 64], f32, tag="state")
            # one-hot inits: fwd rows = e_START, bwd rows = e_STOP
            nc.gpsimd.affine_select(
                out=state[0:32, :], in_=ones_t[0:32, :], pattern=[[0, 64]],
                compare_op=OP.is_equal, fill=0.0, base=-START,
                channel_multiplier=1)
            nc.gpsimd.affine_select(
                out=state[32:64, :], in_=ones_t[32:64, :], pattern=[[0, 64]],
                compare_op=OP.is_equal, fill=0.0, base=-STOP,
                channel_multiplier=1)

            # ---------------- main scan ----------------
            # Combined chunk layout per local step l (CH steps per chunk):
            #   cols [l*64, l*64+32)   = emissions[:, g*CH + l, :]  (fwd)
            #   cols [l*64+32, l*64+64) = emissions[:, S-1-g*CH-l, :]  (bwd)
            comb = None
            for tau in range(HALF):
                if tau % CH == 0:
                    g = tau // CH
                    comb = ccp.tile([BC, 2 * CH * TAGSET], f32, tag="comb")
                    cv = comb[:].rearrange("b (s u t) -> b s u t",
                                           u=2, t=TAGSET)
                    nc.sync.dma_start(cv[:, :, 0, :],
                                      em_d[:, g * CH:(g + 1) * CH, :])
                    nc.sync.dma_start(
                        cv[:, :, 1, :],
                        em_d[:, S - 1 - g * CH:S - (g + 1) * CH - 1:-1, :])
                    iview = iota_t[:].rearrange("b (l t) -> b l t", t=TAGSET)
                    if gold:
                        oh = ohp.tile([BC, 2 * CH * TAGSET], f32, tag="oh")
                        ov = oh[:].rearrange("b (l u t) -> b l u t",
                                             u=2, t=TAGSET)
                        for u in range(2):
                            if u == 0:
                                tsl = tags_sb[:, g * CH:(g + 1) * CH]
                            else:
                                tsl = tags_sb[:, S - 1 - g * CH:
                                              S - (g + 1) * CH - 1:-1]
                            tbc = tsl.rearrange("b l -> b l ()").to_broadcast(
                                [BC, CH, TAGSET])
                            nc.vector.tensor_tensor(
                                out=ov[:, :, u, :],
                                in0=iview, in1=tbc, op=OP.is_equal)
                        if gold >= 2:
                            scrap = ohp.tile([BC, 2 * CH * TAGSET], f32,
                                             tag="scrap")
                            nc.vector.tensor_mul(scrap[:], oh[:], comb[:])
                            nc.vector.tensor_reduce(
                                acc_e[:, g:g + 1], scrap[:],
                                axis=AX.X, op=OP.add)
                l = tau % CH

                tr_ps = trp.tile([64, 64], f32, tag="trps")
                nc.tensor.transpose(tr_ps[:], comb[:, l * 64:(l + 1) * 64],
                                    ident[:])
                xt = xtp.tile([64, 64], f32, tag="xt")
                nc.scalar.activation(xt[:], tr_ps[:], AF.Exp, bias=negmu[:])

                ps = mmp.tile([64, 64], f32, tag="mm")
                nc.tensor.matmul(ps[:], w[:], state[:], start=True, stop=True)
                nstate = stp.tile([64, 64], f32, tag="state")
                nc.vector.tensor_mul(nstate[:], ps[:], xt[:])
                state = nstate
                if debug and tau == 0:
                    nc.sync.dma_start(dbg["d_st0"], state[:])
                    nc.sync.dma_start(dbg["d_xt0"], xt[:])
                    nc.sync.dma_start(dbg["d_w"], w[:])
                if debug and tau == 130:
                    nc.sync.dma_start(dbg["d_st128"], state[:])

                if (tau + 1) % REN == 0:
                    rec = smp.tile([64, 64], f32, tag="rec")
                    nc.vector.reciprocal(rec[0:1, :], state[0:1, :])
                    nc.vector.reciprocal(rec[32:33, :], state[32:33, :])
                    bc_ps = fip.tile([64, 64], f32, tag="bc")
                    nc.tensor.matmul(bc_ps[0:32, :], ones_t[0:1, 0:32],
                                     rec[0:1, :], start=True, stop=True)
                    nc.tensor.matmul(bc_ps[32:64, :], ones_t[32:33, 0:32],
                                     rec[32:33, :], start=True, stop=True,
                                     tile_position=(32, 32))
                    lg = smp.tile([64, 64], f32, tag="lg")
                    nc.scalar.activation(lg[0:1, :], state[0:1, :], AF.Ln)
                    nc.scalar.activation(lg[32:33, :], state[32:33, :], AF.Ln)
                    nc.vector.tensor_add(offacc[0:1, :], offacc[0:1, :],
                                         lg[0:1, :])
                    nc.vector.tensor_add(offacc[32:33, :], offacc[32:33, :],
                                         lg[32:33, :])
                    rstate = stp.tile([64, 64], f32, tag="state")
                    nc.vector.tensor_mul(rstate[:], state[:], bc_ps[:])
                    state = rstate

            # ---------------- finale ----------------
            # beta_1023 = M^T gamma_1024: bwd-final matmul with weights
            # placed so the output lands on partitions 0-31 (aligned with
            # the fwd state for the elementwise dot).
            wb = cp.tile([64, 64], f32)
            nc.vector.memset(wb[:], 0.0)
            nc.sync.dma_start(wb[32:64, 0:32], w[32:64, 32:64])
            psf = mmp.tile([64, 64], f32, tag="mm")
            nc.tensor.matmul(psf[0:32, :], wb[32:64, 0:32], state[32:64, :],
                             start=True, stop=True)
            zp = smp.tile([64, 64], f32, tag="zp")
            nc.vector.tensor_mul(zp[0:32, :], psf[0:32, :], state[0:32, :])
            zsum = fip.tile([1, 64], f32, tag="zsum")
            nc.tensor.matmul(zsum[0:1, :], ones_t[0:32, 0:1], zp[0:32, :],
                             start=True, stop=True)
            gold_c = cp.tile([BC, 1], f32)
            nc.vector.tensor_reduce(gold_c[:], acc_e[:], axis=AX.X, op=OP.add)
            lz = smp.tile([64, 64], f32, tag="lz")
            nc.scalar.activation(lz[0:1, :], zsum[0:1, :], AF.Ln)
            ob = smp.tile([64, 64], f32, tag="ob")
            nc.sync.dma_start(ob[0:1, :], offacc[32:33, :])
            nc.vector.tensor_add(lz[0:1, :], lz[0:1, :], offacc[0:1, :])
            nc.vector.tensor_add(lz[0:1, :], lz[0:1, :], ob[0:1, :])
            # logZ = lz + MU*S;   nll = logZ - gold
            goldT = fip.tile([1, 64], f32, tag="goldT")
            nc.tensor.transpose(goldT[0:1, :], gold_c[:, 0:1], ident[:])
            nc.vector.tensor_sub(lz[0:1, :], lz[0:1, :], goldT[0:1, :])
            nc.vector.tensor_scalar_add(lz[0:1, :], lz[0:1, :],
                                        float(MU) * S)
            nc.sync.dma_start(nll_d, lz[0:1, :])

    nc.compile()
    return nc


def _get_nc():
    if "nc" not in _CACHE:
        _CACHE["nc"] = _build_nc()
    return _CACHE["nc"]


def kernel(emissions, transitions, tags):
    from concourse.bass_utils import run_bass_kernel_spmd

    em = np.ascontiguousarray(np.asarray(emissions, dtype=np.float32))
    tr = np.ascontiguousarray(np.asarray(transitions, dtype=np.float32))
    tg = np.ascontiguousarray(np.asarray(tags, dtype=np.int32))

    nc = _get_nc()
    in_maps = [
        {
            "emissions": em[c * BC:(c + 1) * BC],
            "tags": tg[c * BC:(c + 1) * BC],
            "transitions": tr,
        }
        for c in range(NCORES)
    ]
    res = run_bass_kernel_spmd(nc, in_maps, list(range(NCORES)))
    nll = np.concatenate([res.results[c]["nll"][0] for c in range(NCORES)])
    t_sc = (tr[tg[:, 1:], tg[:, :-1]].sum(axis=1)
            + tr[tg[:, 0], START] + tr[STOP, tg[:, -1]])
    total = np.sum(nll.astype(np.float64)) - np.sum(t_sc.astype(np.float64))
    return np.array(total, dtype=np.float32)



# revision 2
# speedup vs baseline: 1.4441x; 1.4441x over previous
"""CRF negative-log-likelihood loss (BERT_BiLSTM_CRF) on 8 TRN2 NeuronCores.

v2: K-segment burn-in forward scan (data-parallel over batch, 64 seq/core).

Key ideas vs the 1024-tick meet-in-the-middle baseline:
 - Exploit the exponential forgetting of the CRF forward recursion: split
   each sequence into K=32 segments of G=64 steps and run ALL segments in
   parallel as independent columns, each preceded by an L=32-step burn-in
   that converges the state direction to fp32 accuracy.  Serial chain is
   96 ticks instead of 1024.
 - log Z telescopes into per-segment  ln phi(v_k) - ln phi(u_k)  terms
   (phi = per-column within-block sum), measured with two block-sum
   matmuls + Ln; segment 0 is reset to onehot(START) after burn-in so its
   term is exact.
 - Layout: partition = (g, tag) with 4 segment-groups of 32 tags,
   columns = (a, s, seq) with seg k = g*8 + 2a + s.  Per tick one
   [128x128] @ [128x512] bf16 matmul (block-diag exp(transitions))
   and one DVE multiply by exp(emissions - MU).
 - Emissions arrive via gpsimd casting DMAs (f32 DRAM -> bf16 SBUF,
   halves DMA time), are transposed on the PE via strided moving-input
   [128x128] transposes, and exponentiated PSUM->SBUF by the scalar
   engine (bias -MU) in [128x512] batches.
 - Burn-in ticks reuse the previous segment's already-transposed tiles
   via column-shifted APs (no extra DMA/exp except a tiny side pipeline
   for the j=0 columns).
 - Gold emission score: gpsimd local_scatter builds the one-hot mask,
   DVE multiplies it into the raw bf16 emissions, and the scalar engine's
   activation accumulator does the row reduction; the transition part of
   the gold score is folded in on the host during unshard (tiny).
"""
import numpy as np

TAGSET = 32
START = 30
STOP = 31
B = 512
S = 2048
NCORES = 8
BC = B // NCORES          # 64 sequences per core
K = 32                    # segments per sequence
G = S // K                # 64 steps per segment
L = 32                    # burn-in ticks
TICKS = L + G             # 96
MU = np.float32(4.3226)   # mean log-growth per step (measured offline)

_CACHE = {}
import os
_VARIANT = os.environ.get("K2_VARIANT", "full")


def _build_nc():
    import concourse.bacc as bacc
    import concourse.tile as tile
    from concourse import mybir

    f32 = mybir.dt.float32
    bf16 = mybir.dt.bfloat16
    i32 = mybir.dt.int32
    AF = mybir.ActivationFunctionType
    OP = mybir.AluOpType

    nc = bacc.Bacc("TRN2", target_bir_lowering=False, debug=False,
                   num_devices=NCORES)

    em_d = nc.dram_tensor("emissions", [BC, S, TAGSET], f32,
                          kind="ExternalInput").ap()
    tg_d = nc.dram_tensor("tags", [BC, S], i32, kind="ExternalInput").ap()
    tr_d = nc.dram_tensor("transitions", [TAGSET, TAGSET], f32,
                          kind="ExternalInput").ap()
    nll_d = nc.dram_tensor("nll", [1, BC], f32, kind="ExternalOutput").ap()

    # DRAM views: row = (seg, b) -> partition (s, b) slices of 128
    em_v = em_d.rearrange("b (k t) u -> k b (t u)", k=K)
    tg_v = tg_d.rearrange("b (k t) -> k b t", k=K)

    with tile.TileContext(nc) as tc:
        with (
            tc.tile_pool(name="const", bufs=1) as cp,
            tc.tile_pool(name="ebig", bufs=2) as ebp,
            tc.tile_pool(name="xe", bufs=1) as xep,
            tc.tile_pool(name="oh", bufs=2) as ohp,
            tc.tile_pool(name="scr", bufs=1) as scp,
            tc.tile_pool(name="acc", bufs=2) as acp,
            tc.tile_pool(name="state", bufs=3) as stp,
            tc.tile_pool(name="fin", bufs=1) as fnp,
            tc.tile_pool(name="tp", bufs=2, space="PSUM") as tpp,
            tc.tile_pool(name="mm", bufs=2, space="PSUM") as mmp,
            tc.tile_pool(name="fps", bufs=1, space="PSUM") as fpp,
        ):
            # ---------------- constants ----------------
            ones_b = cp.tile([128, 128], bf16)
            nc.vector.memset(ones_b[:], 1.0)
            ident = cp.tile([128, 128], bf16)
            nc.gpsimd.affine_select(
                out=ident[:], in_=ones_b[:], pattern=[[-1, 128]],
                compare_op=OP.is_equal, fill=0.0, base=0, channel_multiplier=1)

            # block-diag exp(transitions)^T weights [128 x 128] bf16
            wbd = cp.tile([128, 128], bf16)
            nc.vector.memset(wbd[:], 0.0)
            w4 = cp.tile([128, 32], f32)
            trT = tr_d.rearrange("a b -> b a")
            for g in range(4):
                nc.sync.dma_start(w4[32 * g:32 * g + 32, :], trT)
            nc.vector.tensor_scalar_max(w4[:], w4[:], -80.0)
            for g in range(4):
                nc.scalar.activation(wbd[32 * g:32 * g + 32,
                                         32 * g:32 * g + 32],
                                     w4[32 * g:32 * g + 32, :], AF.Exp)

            # STOP weights [32 x 1] bf16 at partitions 96:128
            wstop = cp.tile([128, 1], bf16)
            wstop_f = cp.tile([128, 1], f32)
            nc.sync.dma_start(wstop_f[96:128, :],
                              trT[:, STOP:STOP + 1])
            nc.vector.tensor_scalar_max(wstop_f[96:128, :],
                                        wstop_f[96:128, :], -80.0)
            nc.scalar.activation(wstop[96:128, :], wstop_f[96:128, :], AF.Exp)

            # block-sum weights [128 x 4] bf16: w[p, g] = 1 iff p//32 == g
            blk = cp.tile([128, 4], bf16)
            nc.vector.memset(blk[:], 0.0)
            for g in range(4):
                nc.vector.memset(blk[32 * g:32 * g + 32, g:g + 1], 1.0)
            ones4 = cp.tile([4, 1], f32)
            nc.vector.memset(ones4[:], 1.0)

            # stacked identity [128 x 64] f32 (for gold s-fold matmul)
            ones_f = cp.tile([128, 64], f32)
            nc.vector.memset(ones_f[:], 1.0)
            sident = cp.tile([128, 64], f32)
            nc.gpsimd.affine_select(
                out=sident[0:64, :], in_=ones_f[0:64, :], pattern=[[-1, 64]],
                compare_op=OP.is_equal, fill=0.0, base=0, channel_multiplier=1)
            nc.gpsimd.affine_select(
                out=sident[64:128, :], in_=ones_f[64:128, :],
                pattern=[[-1, 64]], compare_op=OP.is_equal, fill=0.0,
                base=0, channel_multiplier=1)

            negmu = cp.tile([128, 1], f32)
            nc.vector.memset(negmu[:], -float(MU))

            # scatter-index iotas: values i*32 (and i*32-1024) for i<64
            iota_a = cp.tile([128, G], i32)
            nc.gpsimd.iota(iota_a[:], pattern=[[TAGSET, G]], base=0,
                           channel_multiplier=0)
            iota_b = cp.tile([128, G], i32)
            nc.gpsimd.iota(iota_b[:], pattern=[[TAGSET, G]], base=-1024,
                           channel_multiplier=0)

            # ---------------- emissions DMA (casting, gpsimd) -------------
            # Ebig[a]: [128 (s,b) x (4g x 64t x 32u)] bf16, rotated bufs=2;
            # interleaved by the Act engine into Ei[a] (t, g, u) before the
            # PE transposes (HW matmul weights AP must be single-free-dim).

            # side pipeline for j=0 burn-in columns: directly interleaved
            # Bi [64 (b) x (32t x 4g x 32u)] bf16; g=0 window is seg 7's
            # tail (garbage for col k=0, reset later; must be finite).
            bi = cp.tile([64, L * 4 * TAGSET], bf16)
            biv = bi[:].rearrange("p (t g u) -> p t g u", t=L, g=4)
            emf = em_d.rearrange("b s u -> b (s u)")
            if "noxeb" in _VARIANT:
                pass
            else:
                for g in range(4):
                    t0 = (g * 8) * G - L if g > 0 else 7 * G  # g=0: valid
                    nc.gpsimd.dma_start(
                        biv[:, :, g, :],
                        emf[:, t0 * TAGSET:(t0 + L) * TAGSET])

            ei = []
            for a in range(4):
                ei_t = xep.tile([128, G * 4 * TAGSET], bf16, tag=f"ei{a}")
                ei.append(ei_t)

            # Xe_all [128 (g,tag) x (64t x 512c)] bf16, c = a*128 + s*64 + b
            xe = xep.tile([128, G * 512], bf16)
            xev = xe[:].rearrange("p (t c) -> p t c", t=G)

            gold_accs = []
            acc_all = cp.tile([128, 16], f32)

            for a in range(4):
                eb = ebp.tile([128, 4 * G * TAGSET], bf16, tag="ebig")
                ebv = eb[:].rearrange("p (g t u) -> p g (t u)", g=4, t=G)
                for g in range(4):
                    k0 = g * 8 + 2 * a
                    nc.gpsimd.dma_start(ebv[:, g, :], em_v[k0:k0 + 2, :, :])
                # Act interleave: Ei[a][:, t, g, :] <- Ebig[a][:, g, t, :]
                eiv = ei[a][:].rearrange("p (t g u) -> p t g u", t=G, g=4)
                for g in range(4):
                    src = ebv[:, g, :].rearrange("p (t u) -> p t u", t=G)
                    nc.scalar.activation(eiv[:, :, g, :], src, AF.Copy,
                                         bias=0.0)
                # gold: one-hot mask (gpsimd local_scatter) + fused
                # multiply-reduce (DVE)
                for g in range(4) if "nogold" not in _VARIANT else []:
                    k0 = g * 8 + 2 * a
                    tr_t = ohp.tile([128, G], i32, tag="traw")
                    nc.sync.dma_start(tr_t[:], tg_v[k0:k0 + 2, :, :])
                    idx_t = ohp.tile([128, 2 * G], mybir.dt.int16,
                                     tag="idx")
                    nc.vector.tensor_add(idx_t[:, 0:G], iota_a[:], tr_t[:])
                    nc.vector.tensor_add(idx_t[:, G:2 * G], iota_b[:],
                                         tr_t[:])
                    oh = ohp.tile([128, G * TAGSET], bf16, tag="oh")
                    nc.gpsimd.local_scatter(
                        out_ap=oh[:, 0:1024], data_ap=ones_b[:, 0:32],
                        idxs_ap=idx_t[:, 0:32], channels=128,
                        num_elems=1024, num_idxs=32)
                    nc.gpsimd.local_scatter(
                        out_ap=oh[:, 1024:2048], data_ap=ones_b[:, 32:64],
                        idxs_ap=idx_t[:, G + 32:G + 64], channels=128,
                        num_elems=1024, num_idxs=32)
                    scr = scp.tile([128, G * TAGSET], bf16, tag="scr")
                    scr2 = scp.tile([128, G * TAGSET], bf16, tag="scr2")
                    nc.vector.tensor_mul(scr[:], ebv[:, g, :], oh[:])
                    idx16 = len(gold_accs)
                    nc.scalar.activation(scr2[:], scr[:], AF.Copy, bias=0.0,
                                         accum_out=acc_all[:,
                                                          idx16:idx16 + 1])
                    gold_accs.append(idx16)

            # ---------------- transpose + exp -> Xe ----------------
            # produce t in burn-first order: t 32..63 then 0..31
            tq_order = list(range(8, 16)) + list(range(0, 8))
            for tq in tq_order:
                for a in range(4):
                    eiv = ei[a][:].rearrange("p (t gu) -> p t gu", t=G)
                    tp = tpp.tile([128, 512], bf16, tag="tp")
                    for i in range(4):
                        t = 4 * tq + i
                        nc.tensor.transpose(tp[:, 128 * i:128 * (i + 1)],
                                            eiv[:, t, :], ident[:])
                    # exp -> Xe slices [t in quad][a block]
                    ov = xev[:, 4 * tq:4 * tq + 4, 128 * a:128 * (a + 1)]
                    nc.scalar.activation(ov, tp[:], AF.Exp, bias=negmu[:])

            # XeB: [128 x (32t x 64b)] bf16 for j=0 columns during burn-in
            xeb = cp.tile([128, L * 64], bf16)
            xebv = xeb[:].rearrange("p (t c) -> p t c", t=L)
            biv2 = bi[:].rearrange("p (t gu) -> p t gu", t=L)
            if "noxeb" in _VARIANT:
                nc.vector.memset(xeb[:], 0.01)
            for tq in range(4) if "noxeb" not in _VARIANT else []:
                tpb = tpp.tile([128, 512], bf16, tag="tp")
                for i in range(8):
                    t = 8 * tq + i
                    nc.tensor.transpose(tpb[:, 64 * i:64 * (i + 1)],
                                        biv2[:, t, :], ident[0:64, 0:64])
                ovb = xebv[:, 8 * tq:8 * tq + 8, :]
                nc.scalar.activation(ovb, tpb[:], AF.Exp, bias=negmu[:])

            # STOP weighting folded into the last tick's Xe for the last
            # segment's columns (block 3, cols 448:512)
            xw = cp.tile([128, 64], bf16)
            nc.vector.tensor_tensor(
                out=xw[96:128, :], in0=xev[96:128, G - 1, 448:512],
                in1=wstop[96:128, :].to_broadcast([32, 64]), op=OP.mult)

            # ---------------- scan ----------------
            state = stp.tile([128, 512], bf16, tag="state")
            nc.vector.memset(state[:], 1.0)

            a_sb = fnp.tile([4, 512], f32)

            for tau in range(TICKS) if "noscan" not in _VARIANT else []:
                ps = mmp.tile([128, 512], f32, tag="mm")
                nc.tensor.matmul(ps[:], wbd[:], state[:], start=True,
                                 stop=True)
                nstate = stp.tile([128, 512], bf16, tag="state")
                if tau < L:
                    tsrc = L + tau  # previous segment's tail tile
                    # cols j>=1 take (j-1)'s data: col shift -64
                    nc.vector.tensor_mul(nstate[:, 64:512], ps[:, 64:512],
                                         xev[:, tsrc, 0:448])
                    # cols j=0 from the side pipeline
                    nc.vector.tensor_mul(nstate[:, 0:64], ps[:, 0:64],
                                         xebv[:, tau, :])
                else:
                    tsrc = tau - L
                    nc.vector.tensor_mul(nstate[:], ps[:], xev[:, tsrc, :])
                    if tau == TICKS - 1:
                        nc.vector.tensor_mul(nstate[96:128, 448:512],
                                             ps[96:128, 448:512],
                                             xw[96:128, :])
                state = nstate
                if tau == L - 1:
                    # burn-in done: A_k = ln(block col sums); reset col k=0
                    aps = fpp.tile([4, 512], f32, tag="meas")
                    nc.tensor.matmul(aps[:], blk[:], state[:], start=True,
                                     stop=True)
                    nc.scalar.activation(a_sb[:], aps[:], AF.Ln)
                    nc.vector.memset(a_sb[0:1, 0:64], 0.0)
                    nc.gpsimd.affine_select(
                        out=state[0:32, 0:64], in_=ones_b[0:32, 0:64],
                        pattern=[[0, 64]], compare_op=OP.is_equal, fill=0.0,
                        base=-START, channel_multiplier=1)
            # ---------------- finale ----------------
            if "noscan" in _VARIANT:
                outz = fnp.tile([1, 64], f32)
                nc.vector.tensor_copy(outz[:], xev[0:1, 5, 0:64])
                if gold_accs:
                    nc.vector.tensor_sub(outz[:], outz[:],
                                         gold_accs[-1][0:1, :].to_broadcast(
                                             [1, 64]))
                nc.sync.dma_start(nll_d, outz[:])
            else:
                bps = fpp.tile([4, 512], f32, tag="meas")
                nc.tensor.matmul(bps[:], blk[:], state[:], start=True,
                                 stop=True)
                b_sb = fnp.tile([4, 512], f32)
                nc.scalar.activation(b_sb[:], bps[:], AF.Ln)
                nc.vector.tensor_sub(b_sb[:], b_sb[:], a_sb[:])
                csum = fpp.tile([1, 512], f32, tag="csum")
                nc.tensor.matmul(csum[:], ones4[:], b_sb[:], start=True,
                                 stop=True)
                csb = fnp.tile([1, 512], f32)
                nc.vector.tensor_copy(csb[:], csum[:])
                t1 = fnp.tile([1, 256], f32)
                nc.vector.tensor_add(t1[:], csb[0:1, 0:256],
                                     csb[0:1, 256:512])
                t2 = fnp.tile([1, 128], f32)
                nc.vector.tensor_add(t2[:], t1[0:1, 0:128], t1[0:1, 128:256])
                t3 = fnp.tile([1, 64], f32)
                nc.vector.tensor_add(t3[:], t2[0:1, 0:64], t2[0:1, 64:128])
                # gold fold: [1 x 64] = gold_acc^T @ stacked_ident
                out = fnp.tile([1, 64], f32)
                if gold_accs:
                    from concourse import mybir as _mb
                    gacc = fnp.tile([128, 1], f32)
                    nc.vector.tensor_reduce(gacc[:], acc_all[:],
                                            axis=_mb.AxisListType.X,
                                            op=OP.add)
                    gfold = fpp.tile([1, 64], f32, tag="gfold")
                    nc.tensor.matmul(gfold[:], gacc[:], sident[:],
                                     start=True, stop=True)
                    nc.vector.tensor_sub(out[:], t3[:], gfold[:])
                else:
                    nc.vector.tensor_copy(out[:], t3[:])
                nc.vector.tensor_scalar_add(out[:], out[:], float(MU) * S)
                nc.sync.dma_start(nll_d, out[:])

    nc.compile()
    return nc


def _get_nc():
    if "nc" not in _CACHE:
        _CACHE["nc"] = _build_nc()
    return _CACHE["nc"]


def kernel(emissions, transitions, tags):
    from concourse.bass_utils import run_bass_kernel_spmd

    em = np.ascontiguousarray(np.asarray(emissions, dtype=np.float32))
    tr = np.ascontiguousarray(np.asarray(transitions, dtype=np.float32))
    tg = np.ascontiguousarray(np.asarray(tags, dtype=np.int32))

    nc = _get_nc()
    in_maps = [
        {
            "emissions": em[c * BC:(c + 1) * BC],
            "tags": tg[c * BC:(c + 1) * BC],
            "transitions": tr,
        }
        for c in range(NCORES)
    ]
    res = run_bass_kernel_spmd(nc, in_maps, list(range(NCORES)))
    nll = np.concatenate([res.results[c]["nll"][0] for c in range(NCORES)])
    t_sc = (tr[tg[:, 1:], tg[:, :-1]].sum(axis=1)
            + tr[tg[:, 0], START] + tr[STOP, tg[:, -1]])
    total = np.sum(nll.astype(np.float64)) - np.sum(t_sc.astype(np.float64))
    return np.array(total, dtype=np.float32)
